# revision 1
# baseline (speedup 1.0000x reference)
"""Trainium2 Bass kernel for CrossAttention + GroupNorm + Swish (nn_CrossAttention).

Reference computation (per batch element b, xf = x[b] reshaped [C, N]):
    q  = Wq @ xf + bq                       [C, N]
    k  = Wk @ ctx^T + bk                    [C, L]
    v  = Wv @ ctx^T + bv                    [C, L]
    qk = (q^T k) * C^-0.5                   [N, L]
    w  = softmax(qk, axis=-1)
    h  = v @ w^T                            [C, N]
    o  = Wo @ h + bo
    xr = o + xf
    out = swish(groupnorm(xr; 32 groups over (C/32, N)) * gamma + beta)

Sharding: data-parallel over batch B=8 across the 8 NeuronCores (no collectives).

Device algorithm (per core):
  - x chunks resident in SBUF ([128|128|64] x 13824 fp32), updated in place with xr.
  - n-tiles of 384 (36 tiles): all matmuls in fp32r (full PE rate for moving dim >= 256).
  - softmax without max-subtraction (scores are tiny: |qk| <~ 2), with the C^-0.5
    scale folded into Wq/bq on the host.
  - colsum of exp-scores + broadcast across partitions in ONE all-ones matmul.
  - GroupNorm stats via bn_stats/bn_aggr per channel + group-membership matmuls
    (gmap [C,32], gmapT [32,C]) for cross-partition group reduction; bias bo is
    folded analytically (mean_c += bo_c; pass-2 affine absorbs a_c*(bo_c - mu_g)).
  - pass 2: out = Silu(a_c * xr + b_c) as a single ACT op per chunk, in place.
"""
import sys

sys.path.insert(0, "/opt/trn_rl_repo")

import numpy as np

import concourse.tile as tile
from concourse import bacc, mybir
from concourse.bass_utils import run_bass_kernel_spmd

F32 = mybir.dt.float32
F32R = mybir.dt.float32r
AF = mybir.ActivationFunctionType
ALU = mybir.AluOpType

# Problem shapes (hardcoded; harness contract)
B, C, D, L, CTX = 8, 320, 24, 77, 768
N = D * D * D            # 13824 spatial positions
G = 32                   # groupnorm groups
GS = C // G              # 10 channels per group
EPS = 1e-5
N_TILE = 256
NT = N // N_TILE         # 36
CCH = [(0, 128), (128, 128), (256, 64)]   # channel chunks (start, size)
KV_COLS = 768            # padded concat [k0,k1,k2+pad, v0,v1,v2+pad]

# Silu isn't implemented in CoreSim; test.py flips this for simulation runs.
USE_SILU = True

# bias6 columns
BQ, BK, BV, BO, GA, BE = range(6)


def _build():
    nc = bacc.Bacc(trn_type="TRN2", target_bir_lowering=False, debug=False)

    x_d = nc.dram_tensor("x", [C, N], F32, kind="ExternalInput")
    ctx_d = nc.dram_tensor("ctx", [L, CTX], F32, kind="ExternalInput")
    wqt_d = nc.dram_tensor("wqt", [C, C], F32R, kind="ExternalInput")
    wot_d = nc.dram_tensor("wot", [C, C], F32R, kind="ExternalInput")
    wkvt_d = nc.dram_tensor("wkvt", [CTX, KV_COLS], F32, kind="ExternalInput")
    bias6_d = nc.dram_tensor("bias6", [C, 6], F32, kind="ExternalInput")
    gmap_d = nc.dram_tensor("gmap", [C, G], F32, kind="ExternalInput")
    gmapt_d = nc.dram_tensor("gmapt", [G, C], F32, kind="ExternalInput")
    ones77_d = nc.dram_tensor("ones77", [L, L], F32R, kind="ExternalInput")
    ident_d = nc.dram_tensor("ident", [128, 128], F32, kind="ExternalInput")
    out_d = nc.dram_tensor("out", [C, N], F32, kind="ExternalOutput")

    with tile.TileContext(nc) as tc:
        _emit(nc, tc, x_d, ctx_d, wqt_d, wot_d, wkvt_d, bias6_d, gmap_d,
              gmapt_d, ones77_d, ident_d, out_d)
    nc.compile()
    return nc


def _emit(nc, tc, x_d, ctx_d, wqt_d, wot_d, wkvt_d, bias6_d, gmap_d,
          gmapt_d, ones77_d, ident_d, out_d):
    from contextlib import ExitStack

    with ExitStack() as ctx_stack:
        const = ctx_stack.enter_context(tc.tile_pool(name="const", bufs=1))
        xpool = ctx_stack.enter_context(tc.tile_pool(name="xbuf", bufs=1))
        kvres = ctx_stack.enter_context(tc.tile_pool(name="kvres", bufs=1))
        psum = ctx_stack.enter_context(tc.tile_pool(name="psum", bufs=8, space="PSUM"))
        loop = ctx_stack.enter_context(tc.tile_pool(name="loop", bufs=2))
        stats = ctx_stack.enter_context(tc.tile_pool(name="stats", bufs=1))

        def ps_tile(p, f):
            return psum.tile([p, f], F32, tag="mm", name="mm")

        # ---------------- constants ----------------
        wq_sb = [const.tile([cs, C], F32R, tag=f"wq{i}", name=f"wq{i}") for i, (c0, cs) in enumerate(CCH)]
        wo_sb = [const.tile([cs, C], F32R, tag=f"wo{i}", name=f"wo{i}") for i, (c0, cs) in enumerate(CCH)]
        b6_sb = [const.tile([cs, 6], F32, tag=f"b6{i}", name=f"b6{i}") for i, (c0, cs) in enumerate(CCH)]
        gm_sb = [const.tile([cs, G], F32, tag=f"gm{i}", name=f"gm{i}") for i, (c0, cs) in enumerate(CCH)]
        gmt_sb = [const.tile([G, cs], F32, tag=f"gmt{i}", name=f"gmt{i}") for i, (c0, cs) in enumerate(CCH)]
        ones_sb = const.tile([L, L], F32R, tag="ones77", name="ones77")
        id_sb = const.tile([128, 128], F32, tag="ident", name="ident")
        for i, (c0, cs) in enumerate(CCH):
            nc.sync.dma_start(out=wq_sb[i], in_=wqt_d.ap()[c0:c0 + cs, :])
            nc.sync.dma_start(out=wo_sb[i], in_=wot_d.ap()[c0:c0 + cs, :])
            nc.sync.dma_start(out=b6_sb[i], in_=bias6_d.ap()[c0:c0 + cs, :])
            nc.sync.dma_start(out=gm_sb[i], in_=gmap_d.ap()[c0:c0 + cs, :])
            nc.sync.dma_start(out=gmt_sb[i], in_=gmapt_d.ap()[:, c0:c0 + cs])
        nc.sync.dma_start(out=ones_sb, in_=ones77_d.ap())
        nc.sync.dma_start(out=id_sb, in_=ident_d.ap())

        # ---------------- prologue: k, v, vT ----------------
        k_sb = [kvres.tile([cs, L], F32R, tag=f"k{i}", name=f"k{i}") for i, (c0, cs) in enumerate(CCH)]
        vt_sb = [kvres.tile([L, cs], F32R, tag=f"vt{i}", name=f"vt{i}") for i, (c0, cs) in enumerate(CCH)]

        with tc.tile_pool(name="prol", bufs=1) as prol:
            kv_ps = [ps_tile(128, L) for _ in range(6)]
            for j in range(6):
                cj_in = prol.tile([L, 128], F32, tag="cj_in", name="cj_in", bufs=2)
                nc.sync.dma_start(out=cj_in, in_=ctx_d.ap()[:, j * 128:(j + 1) * 128])
                tp = ps_tile(128, L)
                nc.tensor.transpose(tp, cj_in, id_sb[0:L, 0:L])
                cj = prol.tile([128, L], F32, tag="ctxt", name="ctxt", bufs=2)
                nc.scalar.activation(cj, tp, AF.Copy)
                wkv_j = prol.tile([128, KV_COLS], F32, tag="wkv", name="wkv", bufs=1)
                nc.sync.dma_start(out=wkv_j, in_=wkvt_d.ap()[j * 128:(j + 1) * 128, :])
                for m in range(6):
                    nc.tensor.matmul(kv_ps[m], wkv_j[:, m * 128:(m + 1) * 128], cj,
                                     start=(j == 0), stop=(j == 5))
            v_sb = []
            for m in range(6):
                if m < 3:
                    c0, cs = CCH[m]
                    nc.scalar.activation(k_sb[m], kv_ps[m][0:cs, :], AF.Identity,
                                         bias=b6_sb[m][:, BK:BK + 1])
                else:
                    c0, cs = CCH[m - 3]
                    vm = prol.tile([cs, L], F32, tag=f"v{m - 3}", name=f"v{m - 3}")
                    nc.scalar.activation(vm, kv_ps[m][0:cs, :], AF.Identity,
                                         bias=b6_sb[m - 3][:, BV:BV + 1])
                    v_sb.append(vm)
            # vT chunks
            for i, (c0, cs) in enumerate(CCH):
                tp = ps_tile(L, 128)
                nc.tensor.transpose(tp[:, 0:cs], v_sb[i], id_sb[0:cs, 0:cs])
                nc.scalar.activation(vt_sb[i], tp[:, 0:cs], AF.Copy)

        # ---------------- resident x chunks + main loop ----------------
        xb = [xpool.tile([cs, N], F32, tag=f"xb{i}", name=f"xb{i}") for i, (c0, cs) in enumerate(CCH)]
        st_sb = [stats.tile([cs, NT, 6], F32, tag=f"st{i}", name=f"st{i}") for i, (c0, cs) in enumerate(CCH)]
        for it in range(NT):
            n0 = it * N_TILE
            nsl = slice(n0, n0 + N_TILE)
            for i, (c0, cs) in enumerate(CCH):
                nc.sync.dma_start(out=xb[i][:, nsl], in_=x_d.ap()[c0:c0 + cs, nsl])
            # rounded fp32r copies of x tile for the q-proj matmuls
            xq = []
            for i, (c0, cs) in enumerate(CCH):
                xq_i = loop.tile([cs, N_TILE], F32R, tag=f"xq{i}", name=f"xq{i}")
                nc.gpsimd.tensor_copy(xq_i, xb[i][:, nsl])
                xq.append(xq_i)
            # q projection: q[m] = sum_k wq[k][:, m].T @ x[k]
            q_sb = []
            for m, (m0, ms) in enumerate(CCH):
                qp = ps_tile(ms, N_TILE)
                for ki in range(3):
                    nc.tensor.matmul(qp, wq_sb[ki][:, m0:m0 + ms], xq[ki],
                                     start=(ki == 0), stop=(ki == 2))
                qm = loop.tile([ms, N_TILE], F32R, tag=f"q{m}", name=f"q{m}")
                nc.scalar.activation(qm, qp, AF.Identity, bias=b6_sb[m][:, BQ:BQ + 1])
                q_sb.append(qm)
            # scores [77, n] = sum_k k_sb[k].T @ q[k]   (scale already in Wq)
            sp = ps_tile(L, N_TILE)
            for ki in range(3):
                nc.tensor.matmul(sp, k_sb[ki], q_sb[ki], start=(ki == 0), stop=(ki == 2))
            u = loop.tile([L, N_TILE], F32R, tag="u", name="u")
            nc.scalar.activation(u, sp, AF.Exp)
            # colsum of u broadcast to 77 partitions via all-ones lhsT
            cb = ps_tile(L, N_TILE)
            nc.tensor.matmul(cb, ones_sb, u, start=True, stop=True)
            rb = loop.tile([L, N_TILE], F32, tag="rb", name="rb")
            nc.vector.reciprocal(out=rb, in_=cb)
            nc.vector.tensor_mul(u, u.bitcast(F32), rb)
            # h[m] = vT[m].T @ un ; copy to SBUF
            h_sb = []
            for m, (m0, ms) in enumerate(CCH):
                hp = ps_tile(ms, N_TILE)
                nc.tensor.matmul(hp, vt_sb[m], u, start=True, stop=True)
                hm = loop.tile([ms, N_TILE], F32R, tag=f"h{m}", name=f"h{m}")
                nc.scalar.activation(hm, hp, AF.Copy)
                h_sb.append(hm)
            # o projection + residual + stats
            for m, (m0, ms) in enumerate(CCH):
                op = ps_tile(ms, N_TILE)
                for ki in range(3):
                    nc.tensor.matmul(op, wo_sb[ki][:, m0:m0 + ms], h_sb[ki],
                                     start=(ki == 0), stop=(ki == 2))
                xmv = xb[m][:, nsl]
                nc.vector.tensor_add(xmv, op, xmv)
                nc.vector.bn_stats(out=st_sb[m][:, it, :], in_=xmv)

        # ---------------- groupnorm stats ----------------
        sm = ctx_stack.enter_context(tc.tile_pool(name="sm", bufs=1))
        st3 = []
        for i, (c0, cs) in enumerate(CCH):
            mv = sm.tile([cs, 2], F32, tag=f"mv{i}", name=f"mv{i}")
            nc.vector.bn_aggr(out=mv, in_=st_sb[i])
            s3 = sm.tile([cs, 3], F32, tag=f"s3{i}", name=f"s3{i}")
            # mean' = mean + bo ; var ; mean'^2
            nc.vector.tensor_add(s3[:, 0:1], mv[:, 0:1], b6_sb[i][:, BO:BO + 1])
            nc.vector.tensor_copy(s3[:, 1:2], mv[:, 1:2])
            nc.vector.tensor_mul(s3[:, 2:3], s3[:, 0:1], s3[:, 0:1])
            st3.append(s3)
        gp = ps_tile(G, 3)
        for i in range(3):
            nc.tensor.matmul(gp, gm_sb[i], st3[i], start=(i == 0), stop=(i == 2))
        # group stats: mu = s_mean/GS ; var = (s_var + s_mean2)/GS - mu^2
        gs = sm.tile([G, 3], F32, tag="gs", name="gs")
        nc.scalar.activation(gs, gp, AF.Copy)
        mu = sm.tile([G, 1], F32, tag="mu", name="mu")
        nc.scalar.activation(mu, gs[:, 0:1], AF.Copy, scale=1.0 / GS)
        tvar = sm.tile([G, 1], F32, tag="tvar", name="tvar")
        nc.vector.tensor_add(tvar, gs[:, 1:2], gs[:, 2:3])
        mu2 = sm.tile([G, 1], F32, tag="mu2", name="mu2")
        nc.vector.tensor_mul(mu2, mu, mu)
        var = sm.tile([G, 1], F32, tag="var", name="var")
        nc.vector.scalar_tensor_tensor(
            out=var, in0=tvar, scalar=1.0 / GS, in1=mu2,
            op0=ALU.mult, op1=ALU.subtract)
        # rstd = 1/sqrt(var + eps)
        epsb = sm.tile([G, 1], F32, tag="epsb", name="epsb")
        nc.vector.memset(epsb, EPS)
        sd = sm.tile([G, 1], F32, tag="sd", name="sd")
        nc.scalar.activation(sd, var, AF.Sqrt, bias=epsb)
        rstd = sm.tile([G, 1], F32, tag="rstd", name="rstd")
        nc.vector.reciprocal(out=rstd, in_=sd)
        mr = sm.tile([G, 2], F32, tag="mr", name="mr")
        nc.vector.tensor_copy(mr[:, 0:1], mu)
        nc.vector.tensor_copy(mr[:, 1:2], rstd)
        # broadcast back per channel: [cs, 2] = gmapT[m].T @ mr
        ab = []
        for m, (m0, ms) in enumerate(CCH):
            bp = ps_tile(ms, 2)
            nc.tensor.matmul(bp, gmt_sb[m], mr, start=True, stop=True)
            a_m = sm.tile([ms, 1], F32, tag=f"a{m}", name=f"a{m}")
            nc.vector.tensor_mul(a_m, bp[:, 1:2], b6_sb[m][:, GA:GA + 1])
            # b = beta + a*(bo - mu)
            t1 = sm.tile([ms, 1], F32, tag=f"t1{m}", name=f"t1{m}")
            nc.vector.tensor_sub(t1, b6_sb[m][:, BO:BO + 1], bp[:, 0:1])
            t2 = sm.tile([ms, 1], F32, tag=f"t2{m}", name=f"t2{m}")
            nc.vector.tensor_mul(t2, t1, a_m)
            b_m = sm.tile([ms, 1], F32, tag=f"b{m}", name=f"b{m}")
            nc.vector.tensor_add(b_m, b6_sb[m][:, BE:BE + 1], t2)
            ab.append((a_m, b_m))

        # ---------------- pass 2: swish + store ----------------
        for it in range(NT):
            n0 = it * N_TILE
            nsl = slice(n0, n0 + N_TILE)
            for m, (m0, ms) in enumerate(CCH):
                a_m, b_m = ab[m]
                xmv = xb[m][:, nsl]
                if USE_SILU:
                    nc.scalar.activation(xmv, xmv, AF.Silu, bias=b_m, scale=a_m)
                else:
                    # sim fallback: xn*sigmoid(xn) with xn = a*x + b, via
                    #   sig = sigmoid(a*x+b); t = (x*a)*sig; out = (sig*b) + t
                    sgm = loop.tile([ms, N_TILE], F32, tag="sg", name="sg", bufs=1)
                    nc.scalar.activation(sgm, xmv, AF.Sigmoid, bias=b_m, scale=a_m)
                    nc.vector.scalar_tensor_tensor(
                        out=xmv, in0=xmv, scalar=a_m, in1=sgm,
                        op0=ALU.mult, op1=ALU.mult)
                    nc.vector.scalar_tensor_tensor(
                        out=xmv, in0=sgm, scalar=b_m, in1=xmv,
                        op0=ALU.mult, op1=ALU.add)
                nc.sync.dma_start(out=out_d.ap()[m0:m0 + ms, nsl], in_=xb[m][:, nsl])


_NC_CACHE = None


def _get_nc():
    global _NC_CACHE
    if _NC_CACHE is None:
        _NC_CACHE = _build()
    return _NC_CACHE


def _host_consts(Wq, bq, Wk, bk, Wv, bv, Wo, bo, gamma, beta):
    s = float(C) ** -0.5
    wqt = np.ascontiguousarray((Wq * s).T.astype(np.float32))
    wot = np.ascontiguousarray(Wo.T.astype(np.float32))
    wkvt = np.zeros((CTX, KV_COLS), np.float32)
    wkt = Wk.T.astype(np.float32)   # [CTX, C]
    wvt = Wv.T.astype(np.float32)
    wkvt[:, 0:128] = wkt[:, 0:128]
    wkvt[:, 128:256] = wkt[:, 128:256]
    wkvt[:, 256:320] = wkt[:, 256:320]
    wkvt[:, 384:512] = wvt[:, 0:128]
    wkvt[:, 512:640] = wvt[:, 128:256]
    wkvt[:, 640:704] = wvt[:, 256:320]
    bias6 = np.stack([bq * s, bk, bv, bo, gamma, beta], axis=1).astype(np.float32)
    bias6 = np.ascontiguousarray(bias6)
    gmap = np.zeros((C, G), np.float32)
    gmap[np.arange(C), np.arange(C) // GS] = 1.0
    gmapt = np.ascontiguousarray(gmap.T)
    ones77 = np.ones((L, L), np.float32)
    ident = np.eye(128, dtype=np.float32)
    return dict(wqt=wqt, wot=wot, wkvt=wkvt, bias6=bias6, gmap=gmap,
                gmapt=gmapt, ones77=ones77, ident=ident)


def kernel(x, context, Wq, bq, Wk, bk, Wv, bv, Wo, bo, gamma, beta,
           _return_results=False, _trace=False):
    x = np.asarray(x, np.float32)
    context = np.asarray(context, np.float32)
    consts = _host_consts(np.asarray(Wq, np.float32), np.asarray(bq, np.float32),
                          np.asarray(Wk, np.float32), np.asarray(bk, np.float32),
                          np.asarray(Wv, np.float32), np.asarray(bv, np.float32),
                          np.asarray(Wo, np.float32), np.asarray(bo, np.float32),
                          np.asarray(gamma, np.float32), np.asarray(beta, np.float32))
    nc = _get_nc()
    in_maps = []
    for b in range(B):
        m = dict(consts)
        m["x"] = np.ascontiguousarray(x[b].reshape(C, N))
        m["ctx"] = np.ascontiguousarray(context[b])
        in_maps.append(m)
    res = run_bass_kernel_spmd(nc, in_maps, core_ids=list(range(B)), trace=_trace)
    out = np.stack([res.results[b]["out"].reshape(C, D, D, D) for b in range(B)])
    if _return_results:
        return out, res
    return out

